# revision 56
# baseline (speedup 1.0000x reference)
"""Trainium2 Bass kernel: MultiHeadCrossAttentionWithBias.

Reference computation (per batch b):
  q_u = scale*(u_enc @ wq + wq_b); k/v from e_enc (and vice versa)
  ue_w = softmax(q_u k_e^T + bpp + mask*-inf); u_ctx = ue_w @ v_e
  u_update = u_ctx @ wo + wo_b                     (same mirrored for e)

Sharding: the problem decomposes into 8 fully independent attention units:
(batch b, direction d) for b in 0..3, d in {u->e, e->u}. Core i = (d, b)
handles one unit end-to-end; no collectives needed.

Host prep is layout/precision only (transposes, slices, fp32->bf16
rounding of device operands); all FLOPs run on device.

Per-core inputs (all bf16 except biases/scalars):
  encQT  [D=512, L=1024]  query-side encoder, transposed
  encKT  [D=512, L=1024]  key-side encoder, transposed
  bpp    [L, L]           logit bias oriented [k, q]
  mask   [L, L]           mask oriented [k, q], {0,1}
  wq/wk/wv [D, 512], wo [512, D]; biases f32

On-device math (per core), all matmuls bf16 with fp32 PSUM accumulate:
  qT[f, s] = scale*(wq^T encQT + wq_b)   (f = h*64+hd on partitions)
  kT[f, s] =        wk^T encKT + wk_b
  v[s, f]  =        encKT^T wv + wv_b    (+ fused ones column per head)
  Ecb[k, q] = exp(bpp_w*bpp + bpp_b) * mask   (exact 0 at masked slots;
              folds the softmax bias AND the post-softmax re-mask)
  per head h, k-chunk kc:
      S^T[k,q] = kT_h^T qT_h           (PE -> PSUM fp32)
      E0 = exp(S^T)                    (ACT -> bf16; logits O(10), no max)
      E = E0 * Ecb[kc]                 (DVE, bf16 in-place)
      [ctx^T; den] += [v_h | 1]^T E    (PE, PSUM accumulation over kc)
  rcp = reciprocal_approx_fast(den); bf16; broadcast via DRAM bounce
  ctxn[pair] = ctx^T * rcp  (DVE bf16, odd head on partitions 64..127)
  out[s, e] = sum_pair ctxn_p^T wo_p + wo_b   (PE + DVE bias-add eviction)

Engine budget per core (steady state): ACT carries only the 64 exps
(~1.3us each); PE carries 384 bf16 matmuls (~220ns each); DVE carries
masking/evictions/normalize; attention phase runs at the ACT exp rate.
"""

import numpy as np
import ml_dtypes
from contextlib import ExitStack

import concourse.bass as bass
import concourse.tile as tile
import concourse.bacc as bacc
import concourse.mybir as mybir
from concourse import bass_utils

F32 = mybir.dt.float32
BF16 = mybir.dt.bfloat16
AF = mybir.ActivationFunctionType
ALU = mybir.AluOpType

B, L, D, H, HD = 4, 1024, 512, 8, 64
P = 128
FH = H * HD            # 512
SCALE = 1.0 / np.sqrt(HD)
N_CORES = 8
LAG = 4
DUMMY_COLS = 256


def bcast_ap(dram_ap, parts):
    """Partition-step-0 broadcast AP over a DRAM row."""
    return bass.AP(tensor=dram_ap.tensor, offset=dram_ap.offset,
                   ap=[[0, parts]] + list(dram_ap.ap))


def build_module():
    nc = bacc.Bacc("TRN2", target_bir_lowering=False, debug=False)

    encQT_d = nc.dram_tensor("encQT", [D, L], BF16, kind="ExternalInput")
    encKT_d = nc.dram_tensor("encKT", [D, L], BF16, kind="ExternalInput")
    wq_d = nc.dram_tensor("wq", [D, FH], BF16, kind="ExternalInput")
    wk_d = nc.dram_tensor("wk", [D, FH], BF16, kind="ExternalInput")
    wv_d = nc.dram_tensor("wv", [D, FH], BF16, kind="ExternalInput")
    wo_d = nc.dram_tensor("wo", [FH, D], BF16, kind="ExternalInput")
    bpp_d = nc.dram_tensor("bpp", [L, L], F32, kind="ExternalInput")
    mask_d = nc.dram_tensor("mask", [L, L], BF16, kind="ExternalInput")
    wqb_d = nc.dram_tensor("wqb", [FH], F32, kind="ExternalInput")
    wkb_d = nc.dram_tensor("wkb", [FH], F32, kind="ExternalInput")
    wvb_d = nc.dram_tensor("wvb", [FH], F32, kind="ExternalInput")
    wob_d = nc.dram_tensor("wob", [D], F32, kind="ExternalInput")
    bppw_d = nc.dram_tensor("bppw", [1, 1], F32, kind="ExternalInput")
    bppb_d = nc.dram_tensor("bppb", [1, 1], F32, kind="ExternalInput")
    out_d = nc.dram_tensor("out", [L, D], BF16, kind="ExternalOutput")
    den_d = nc.dram_tensor("den_scratch", [H, L], BF16, kind="Internal")

    with tile.TileContext(nc) as tc, ExitStack() as ctx:
        const = ctx.enter_context(tc.tile_pool(name="const", bufs=1))
        qkT_p = ctx.enter_context(tc.tile_pool(name="qkT", bufs=8))
        v_p = ctx.enter_context(tc.tile_pool(name="v", bufs=8))
        wo_p = ctx.enter_context(tc.tile_pool(name="wo", bufs=4))
        ecb_p = ctx.enter_context(tc.tile_pool(name="ecb", bufs=8))
        enc_p = ctx.enter_context(tc.tile_pool(name="enc", bufs=8))
        w_p = ctx.enter_context(tc.tile_pool(name="wqkv", bufs=12))
        ps_s = tc.alloc_tile_pool(name="ps_s", bufs=3, space="PSUM")
        ps_c = tc.alloc_tile_pool(name="ps_c", bufs=2, space="PSUM")

        # ---- small bias prep (tiny DMAs) ----
        bw_col = const.tile([P, 1], F32)
        nc.gpsimd.dma_start(bw_col[:], bcast_ap(bppw_d.ap()[0:1, :], P))
        bb_col = const.tile([P, 1], F32)
        nc.gpsimd.dma_start(bb_col[:], bcast_ap(bppb_d.ap()[0:1, :], P))
        wqb_raw = const.tile([P, 4], F32)
        nc.gpsimd.dma_start(wqb_raw[:], wqb_d.ap().rearrange("(c p) -> p c", p=P))
        wqb_sc = const.tile([P, 4], F32)
        nc.vector.tensor_scalar_mul(wqb_sc[:], wqb_raw[:], float(SCALE))
        wkb_c = const.tile([P, 4], F32)
        nc.gpsimd.dma_start(wkb_c[:], wkb_d.ap().rearrange("(c p) -> p c", p=P))
        wvb_bc = const.tile([P, FH], F32)
        nc.gpsimd.dma_start(wvb_bc[:], bcast_ap(wvb_d.ap(), P))
        wob_bc = const.tile([P, D], F32)
        nc.gpsimd.dma_start(wob_bc[:], bcast_ap(wob_d.ap(), P))
        # persistent denominator-reciprocal scratch: valid rows at 0/32/64/96
        # are rewritten each head pair; the rest stay at the memset value so
        # the full-tile bf16 convert below never reads garbage.
        rcp97 = const.tile([97, 512], F32)
        den97 = const.tile([97, 512], F32)
        nc.vector.memset(den97[:], 1.0)
        # output bias as a rank-1 matmul term: ones[1,128]^T @ wob[1,512]
        ones1 = const.tile([1, P], BF16)
        nc.vector.memset(ones1[:], 1.0)
        wob_bf = const.tile([1, D], BF16)
        nc.vector.tensor_copy(wob_bf[:], wob_bc[0:1, :])

        # ---- bulk loads, first-use order; triggers spread across engine
        # queues (a DMA trigger costs ~660ns of queue time, so serializing
        # them all on sync would dominate the prolog).
        eq, ek = [], []
        wq_t, wk_t, wv_t = [], [], []
        for w_dram, wlst, elst, edram, eng in (
            (wq_d, wq_t, eq, encQT_d, nc.sync),
            (wk_d, wk_t, ek, encKT_d, nc.scalar),
            (wv_d, wv_t, None, None, nc.sync),
        ):
            for dc in range(4):
                t = w_p.tile([P, FH], BF16, tag="w", name=f"w_{w_dram.name}{dc}")
                eng.dma_start(t[:], w_dram.ap()[dc * P:(dc + 1) * P, :])
                wlst.append(t)
            if elst is None:
                continue
            for dc in range(4):
                t = enc_p.tile([P, L], BF16, tag="enc",
                               name=f"enc_{edram.name}{dc}")
                eng.dma_start(t[:], edram.ap()[dc * P:(dc + 1) * P, :])
                elst.append(t)

        # ---- wo loads (late: not projection-critical) ----
        wo_t = []
        for p_ in range(4):
            t = wo_p.tile([P, D], BF16, tag="wo", name=f"wo{p_}")
            nc.scalar.dma_start(t[:], wo_d.ap()[p_ * P:(p_ + 1) * P, :])
            wo_t.append(t)

        # ---- Ecb[k, q] = exp(bpp_w*bpp + bpp_b) * mask  (bf16) ----
        # DMA emitted after projection-critical loads; ACT/DVE work overlaps
        # the projection matmuls. Masked entries become exact 0, which also
        # implements the reference's post-softmax re-mask.
        ecb = []
        ebt_p = tc.alloc_tile_pool(name="ebtmp", bufs=3)
        for kc in range(8):
            b_t = ebt_p.tile([P, L], F32, tag="b", name=f"b{kc}")
            nc.gpsimd.dma_start(b_t[:], bpp_d.ap()[kc * P:(kc + 1) * P, :])
            m_t = ebt_p.tile([P, L], BF16, tag="m", name=f"m{kc}")
            nc.gpsimd.dma_start(m_t[:], mask_d.ap()[kc * P:(kc + 1) * P, :])
            eb_t = ebt_p.tile([P, L], BF16, tag="eb", name=f"eb{kc}")
            nc.scalar.activation(eb_t[:], b_t[:], AF.Exp,
                                 bias=bb_col[:, 0:1], scale=bw_col[:, 0:1])
            c_t = ecb_p.tile([P, L], BF16, tag="ecb", name=f"ecb{kc}")
            nc.vector.scalar_tensor_tensor(c_t[:], eb_t[:], 1.0, m_t[:],
                                           ALU.bypass, ALU.mult)
            ecb.append(c_t)
        ebt_p.release()

        qT, kT = [None] * 4, [None] * 4

        def emit_proj_half(pc, which):
            """qT[pc] or kT[pc]: [f, s] packed two heads per 128-part chunk.

            Uniform [128, L] PSUM tiles (same shape as score tiles) so one
            pool serves projections and attention without mixed-size
            allocation; one eviction per pc instead of two."""
            w_t, enc_t, out_list = ((wq_t, eq, qT) if which == "q"
                                    else (wk_t, ek, kT))
            o = qkT_p.tile([P, L], BF16, tag="qkT", name=f"{which}T{pc}")
            ps = ps_s.tile([P, L], F32, tag="ps_s", name=f"ps_{which}{pc}")
            for sh in range(2):
                for dc in range(4):
                    nc.tensor.matmul(
                        ps[:, sh * 512:(sh + 1) * 512],
                        w_t[dc][:, pc * P:(pc + 1) * P],
                        enc_t[dc][:, sh * 512:(sh + 1) * 512],
                        start=(dc == 0), stop=(dc == 3))
            if which == "q":
                nc.vector.tensor_scalar(
                    o[:], ps[:], float(SCALE),
                    wqb_sc[:, pc:pc + 1], ALU.mult, ALU.add)
            else:
                nc.vector.tensor_scalar_add(
                    o[:], ps[:], wkb_c[:, pc:pc + 1])
            out_list[pc] = o

        v_aug = []

        def emit_v_pair(sc2):
            """v: [s, f] with ones column interleaved per head ([128, 8*65])."""
            ps = ps_s.tile([P, L], F32, tag="ps_s", name=f"ps_v{sc2}")
            for j in range(2):
                sc = 2 * sc2 + j
                for dc in range(4):
                    nc.tensor.matmul(
                        ps[:, j * 512:(j + 1) * 512],
                        ek[dc][:, sc * P:(sc + 1) * P],
                        wv_t[dc][:], start=(dc == 0), stop=(dc == 3))
            for j in range(2):
                sc = 2 * sc2 + j
                va = v_p.tile([P, H * (HD + 1)], BF16, tag="v", name=f"v{sc}")
                vg = va[:].rearrange("p (h c) -> p h c", c=HD + 1)
                nc.vector.scalar_tensor_tensor(
                    vg[:, :, 0:HD],
                    ps[:, j * 512:(j + 1) * 512].rearrange(
                        "p (h c) -> p h c", c=HD), 1.0,
                    wvb_bc[:].rearrange("p (h c) -> p h c", c=HD),
                    ALU.bypass, ALU.add)
                nc.vector.memset(vg[:, :, HD:HD + 1], 1.0)
                v_aug.append(va)

        # ---- attention ----
        ctxn_p = ctx.enter_context(tc.tile_pool(name="ctxn", bufs=4))
        ctxr_p = ctx.enter_context(tc.tile_pool(name="ctxr", bufs=2))
        den_p = ctx.enter_context(tc.tile_pool(name="den", bufs=2))
        e_p = ctx.enter_context(tc.tile_pool(name="e", bufs=6))
        rb_p = ctx.enter_context(tc.tile_pool(name="rb", bufs=2))
        ctxn = [None] * 4
        pair_state = {}

        def emit_head(h, fillers=(), dummies=()):
            """fillers: {kc: thunk} of PE work emitted inside this head's
            stream; dummies: kc positions where one discarded 512-col matmul
            keeps the PE queue nonempty (a PE idle of >~1.5us drops the clock
            ratchet, which rarely recovers; <1us gaps are survivable)."""
            lag = LAG + 1 if h == 0 else LAG
            o = (h % 2) * HD
            pc = h // 2
            c_ps = [ps_c.tile([HD + 1, 512], F32, tag="ps_c",
                              name=f"c_ps_{h}_{i}") for i in range(2)]
            e_ts = {}
            for kc in range(8 + lag):
                if kc in fillers:
                    fillers[kc]()
                if kc < 8:
                    s_ps = ps_s.tile([P, L], F32, tag="ps_s",
                                     name=f"s_ps_{h}_{kc}")
                    if kc in dummies:
                        nc.tensor.matmul(
                            s_ps[:, 0:DUMMY_COLS],
                            kT[pc][o:o + HD, 0:P],
                            qT[pc][o:o + HD, 0:DUMMY_COLS],
                            start=True, stop=True, skip_group_check=True)
                    for qh in range(2):
                        sl = slice(qh * 512, (qh + 1) * 512)
                        nc.tensor.matmul(
                            s_ps[:, sl],
                            kT[pc][o:o + HD, kc * P:(kc + 1) * P],
                            qT[pc][o:o + HD, sl],
                            start=True, stop=True)
                    et = e_p.tile([P, L], BF16, tag="e", name=f"e_{h}_{kc}")
                    nc.scalar.activation(et[:], s_ps[:], AF.Exp)
                    # mask+bias multiply: plain tensor_tensor hits the DVE
                    # 2x bf16 mode (~830ns) where STT runs full-rate (~1.45us);
                    # gpsimd is avoided entirely — its tensor ops contend for
                    # the shared SBUF ports and slow concurrent DVE ops ~4x.
                    nc.vector.tensor_mul(et[:], et[:], ecb[kc][:])
                    e_ts[kc] = et
                if kc >= lag:
                    kp = kc - lag
                    for qh in range(2):
                        sl = slice(qh * 512, (qh + 1) * 512)
                        nc.tensor.matmul(
                            c_ps[qh][:],
                            v_aug[kp][:, h * (HD + 1):(h + 1) * (HD + 1)],
                            e_ts[kp][:, sl],
                            start=(kp == 0), stop=(kp == 7))
            # evict raw ctx (bf16); reciprocal the denominator rows straight
            # from PSUM into a persistent 32-row-strided scratch tile.
            if h % 2 == 0:
                ctxn[pc] = ctxn_p.tile([P, L], BF16, tag="ctxn",
                                       name=f"ctxn{pc}")
                ctxr = ctxr_p.tile([P, L], BF16, tag="ctxr", name=f"ctxr{pc}")
                pair_state["hold"] = ctxr
            else:
                ctxr = pair_state["hold"]
            # denominator rows first so the reciprocal chain starts ASAP
            for qh in range(2):
                r_ = ((h % 2) * 2 + qh) * 32
                nc.vector.tensor_copy(den97[r_:r_ + 1, :],
                                      c_ps[qh][HD:HD + 1, :])
            dflat = den_d.ap()[2 * pc:2 * pc + 2, :].rearrange(
                "h (a b) -> (h a) b", b=512)
            if h == 6:
                # last pair only: resolve head 6's reciprocal/broadcast half
                # during head 7's iterations, so the tail critical path holds
                # just head 7's half (custom DVE ops need partition-0 starts,
                # hence full-range recip; the extra rows are idempotent).
                nc.vector.reciprocal_approx_fast(rcp97[:], den97[:])
                rcpb6 = den_p.tile([97, 512], BF16, tag="rcpb", name="rcpb6")
                nc.vector.tensor_copy(rcpb6[:], rcp97[:])
                for r_ in range(2):
                    nc.sync.dma_start(dflat[r_:r_ + 1, :],
                                      rcpb6[32 * r_:32 * r_ + 1, :])
                rb = rb_p.tile([P, L], BF16, tag="rb", name=f"rb{pc}")
                pair_state["rb"] = rb
                nc.gpsimd.dma_start(
                    rb[0:HD, :],
                    bcast_ap(den_d.ap()[2 * pc:2 * pc + 1, :], HD))
            for qh in range(2):
                sl = slice(qh * 512, (qh + 1) * 512)
                nc.vector.tensor_copy(ctxr[o:o + HD, sl], c_ps[qh][0:HD, :])
            if h % 2 == 1:
                nc.vector.reciprocal_approx_fast(rcp97[:], den97[:])
                rcpb = den_p.tile([97, 512], BF16, tag="rcpb", name=f"rcpb{pc}")
                nc.vector.tensor_copy(rcpb[:], rcp97[:])
                if h == 7:
                    rb = pair_state["rb"]
                    for r_ in range(2, 4):
                        nc.sync.dma_start(dflat[r_:r_ + 1, :],
                                          rcpb[32 * r_:32 * r_ + 1, :])
                    nc.gpsimd.dma_start(
                        rb[HD:P, :],
                        bcast_ap(den_d.ap()[2 * pc + 1:2 * pc + 2, :], HD))
                else:
                    for r_ in range(4):
                        nc.sync.dma_start(dflat[r_:r_ + 1, :],
                                          rcpb[32 * r_:32 * r_ + 1, :])
                    rb = rb_p.tile([P, L], BF16, tag="rb", name=f"rb{pc}")
                    nc.gpsimd.dma_start(
                        rb[0:HD, :],
                        bcast_ap(den_d.ap()[2 * pc:2 * pc + 1, :], HD))
                    nc.gpsimd.dma_start(
                        rb[HD:P, :],
                        bcast_ap(den_d.ap()[2 * pc + 1:2 * pc + 2, :], HD))
                nc.vector.tensor_mul(ctxn[pc][:], ctxr[:], rb[:])

        # The PE clock ratchets down on any substantial idle and does not
        # recover, so after the prolog the PE stream must never starve:
        # pc0 q/k + the first v pairs run up front, the remaining projection
        # chains are spread INSIDE heads 0-2 as filler (the attention alone
        # leaves the PE idle while ACT exps catch up), and later heads get
        # one dummy matmul per iteration as pacing filler.
        allkc = tuple(range(8))
        odd = (1, 3, 5, 7)
        emit_proj_half(0, "q")
        emit_proj_half(0, "k")
        emit_v_pair(0)
        emit_v_pair(1)
        emit_head(0, fillers={
            0: lambda: emit_v_pair(2),
            2: lambda: emit_v_pair(3),
            4: lambda: emit_proj_half(1, "q"),
            6: lambda: emit_proj_half(1, "k"),
        }, dummies=odd)
        emit_head(1, fillers={
            1: lambda: emit_proj_half(2, "q"),
            4: lambda: emit_proj_half(2, "k"),
        }, dummies=odd)
        emit_head(2, fillers={
            1: lambda: emit_proj_half(3, "q"),
            4: lambda: emit_proj_half(3, "k"),
        }, dummies=odd)
        for h in range(3, H):
            emit_head(h, dummies=allkc)
        # pacing filler across the last pair's epilogue so the clock holds
        # into the output projection
        for i_ in range(16):
            s_ps = ps_s.tile([P, L], F32, tag="ps_s", name=f"s_pad{i_}")
            nc.tensor.matmul(s_ps[:, 0:512], kT[3][64:P, 0:P],
                             qT[3][64:P, 0:512],
                             start=True, stop=True, skip_group_check=True)

        # ---- output projection ----
        # p-major emission: all pair-0 matmuls first, so the PE only waits
        # on the last pair's normalize chain for the final 8 matmuls.
        ps_c.release()
        ps_s.release()
        ps_o = tc.alloc_tile_pool(name="ps_o", bufs=8, space="PSUM")
        with tc.tile_pool(name="outp", bufs=3) as out_p:
            o_ps = [ps_o.tile([P, D], F32, tag="ps_o", name=f"o_ps{st}")
                    for st in range(8)]
            for p_ in range(4):
                for st in range(8):
                    nc.tensor.matmul(o_ps[st][:],
                                     ctxn[p_][:, st * P:(st + 1) * P],
                                     wo_t[p_][:],
                                     start=(p_ == 0), stop=False)
            out_engs = [nc.sync, nc.scalar, nc.gpsimd, nc.sync]
            for st in range(8):
                nc.tensor.matmul(o_ps[st][:], ones1[:], wob_bf[:],
                                 start=False, stop=True)
                o_t = out_p.tile([P, D], BF16, tag="out", name=f"out{st}")
                nc.scalar.copy(o_t[:], o_ps[st][:])
                for s_ in range(2):
                    rs = slice(st * P + s_ * 64, st * P + (s_ + 1) * 64)
                    out_engs[(2 * st + s_) % 4].dma_start(
                        out_d.ap()[rs, :], o_t[s_ * 64:(s_ + 1) * 64, :])
        ps_o.release()

    nc.compile()
    return nc


def to_bf16(x):
    return np.asarray(x, np.float32).astype(ml_dtypes.bfloat16)


def shard_inputs(u_enc, e_enc, logit_bpp, ue_mask, eu_mask,
                 wq_k, wq_b, wk_k, wk_b, wv_k, wv_b, wo_k, wo_b,
                 bpp_w, bpp_b):
    """Build the 8 per-core input maps (layout + bf16 rounding only)."""
    u_enc = np.asarray(u_enc, np.float32)
    e_enc = np.asarray(e_enc, np.float32)
    bpp = np.asarray(logit_bpp, np.float32)
    ue_m = np.asarray(ue_mask).astype(np.float32)
    eu_m = np.asarray(eu_mask).astype(np.float32)
    com = dict(
        wq=to_bf16(np.asarray(wq_k, np.float32).reshape(D, FH)),
        wk=to_bf16(np.asarray(wk_k, np.float32).reshape(D, FH)),
        wv=to_bf16(np.asarray(wv_k, np.float32).reshape(D, FH)),
        wo=to_bf16(np.asarray(wo_k, np.float32).reshape(FH, D)),
        wqb=np.asarray(wq_b, np.float32).reshape(FH).copy(),
        wkb=np.asarray(wk_b, np.float32).reshape(FH).copy(),
        wvb=np.asarray(wv_b, np.float32).reshape(FH).copy(),
        wob=np.asarray(wo_b, np.float32).reshape(D).copy(),
        bppw=np.asarray(bpp_w, np.float32).reshape(1, 1).copy(),
        bppb=np.asarray(bpp_b, np.float32).reshape(1, 1).copy(),
    )
    uT = [to_bf16(u_enc[b].T) for b in range(B)]
    eT = [to_bf16(e_enc[b].T) for b in range(B)]
    bppT = np.ascontiguousarray(bpp.T)
    in_maps = []
    for i in range(N_CORES):
        d, b = divmod(i, B)
        if d == 0:      # u queries, e keys -> u_update[b]
            m = dict(encQT=uT[b], encKT=eT[b], bpp=bppT,
                     mask=to_bf16(ue_m[b, 0].T))
        else:           # e queries, u keys -> e_update[b]
            m = dict(encQT=eT[b], encKT=uT[b], bpp=bpp,
                     mask=to_bf16(eu_m[b, 0].T))
        m.update(com)
        in_maps.append(m)
    return in_maps


_NC = None


def kernel(**inputs):
    global _NC
    if _NC is None:
        _NC = build_module()
    in_maps = shard_inputs(**inputs)
    res = bass_utils.run_bass_kernel_spmd(
        _NC, in_maps, core_ids=list(range(N_CORES)))
    u_update = np.stack([np.asarray(res.results[b]["out"]).astype(np.float32)
                         for b in range(B)])
    e_update = np.stack(
        [np.asarray(res.results[B + b]["out"]).astype(np.float32)
         for b in range(B)])
    return u_update, e_update


if __name__ == "__main__":
    # single-core CoreSim check of one (direction, batch) unit
    from concourse.bass_interp import CoreSim

    rng = np.random.default_rng(0)
    u = rng.standard_normal((B, L, D)).astype(np.float32)
    e = rng.standard_normal((B, L, D)).astype(np.float32)
    bpp = rng.standard_normal((L, L)).astype(np.float32)
    uem = (rng.random((B, 1, L, L)) < 0.9)
    eum = (rng.random((B, 1, L, L)) < 0.9)
    w = 1.0 / np.sqrt(D)
    wq = (rng.standard_normal((D, H, HD)) * w).astype(np.float32)
    wk = (rng.standard_normal((D, H, HD)) * w).astype(np.float32)
    wv = (rng.standard_normal((D, H, HD)) * w).astype(np.float32)
    wo = (rng.standard_normal((H, HD, D)) / np.sqrt(FH)).astype(np.float32)
    zq = (rng.standard_normal((H, HD)) * 0.1).astype(np.float32)
    zo = (rng.standard_normal((D,)) * 0.1).astype(np.float32)

    nc = build_module()
    in_maps = shard_inputs(u, e, bpp, uem, eum, wq, zq, wk, zq, wv, zq,
                           wo, zo, np.float32(1.3), np.float32(-0.2))

    core = 0
    sim = CoreSim(nc, trace=False)
    for k, vv in in_maps[core].items():
        sim.tensor(k)[:] = vv
    sim.simulate(check_with_hw=False)
    got = np.array(sim.tensor("out")).astype(np.float32)
    print("sim time estimate:", sim.time, "ns")

    def ref_unit(encQ, encK, bias_qk, mask_qk):
        q = SCALE * (encQ @ wq.reshape(D, FH) + zq.reshape(FH))
        kk = encK @ wk.reshape(D, FH) + zq.reshape(FH)
        vv = encK @ wv.reshape(D, FH) + zq.reshape(FH)
        accum = np.zeros((L, D), np.float64)
        for h in range(H):
            qi = q[:, h * HD:(h + 1) * HD]
            ki = kk[:, h * HD:(h + 1) * HD]
            vi = vv[:, h * HD:(h + 1) * HD]
            s = qi @ ki.T + bias_qk
            s = np.where(mask_qk, s, -np.inf)
            s = s - s.max(-1, keepdims=True)
            p_ = np.exp(s)
            p_ /= p_.sum(-1, keepdims=True)
            accum += (p_ @ vi) @ wo[h]
        return (accum + zo).astype(np.float32)

    bq = 1.3 * bpp + -0.2
    exp_out = ref_unit(u[0], e[0], bq, uem[0, 0])
    err = np.abs(got - exp_out).max() / np.abs(exp_out).max()
    print("unit relerr vs numpy:", err)
